# revision 16
# baseline (speedup 1.0000x reference)
"""Trainium2 Bass kernel for nn_Attn: attn = softmax(enc @ W^T @ hidden^T).

Math: reference computes energy = enc @ W^T + b  ([S,H]), then
attn_energies = energy @ hidden[0]  ([S]), then softmax over S.
Associativity: attn_energies = enc @ (W^T @ hidden^T) + (b . hidden).
The (b . hidden) term is a constant shift over S -> softmax-invariant
(and b is zeros for this problem), so we drop it.

Distribution over 8 cores = 2 row-groups x 4 column-groups:
  core r -> row-group g = r // 4 (8192 seq rows), col-group c = r % 4
  (512 hidden columns).
  - Each core computes u_c = hidden @ W[:, c-shard] on the PE (purely
    local; W shard is 4MB) and then partial energies
    e_r[s] = enc[s, c-shard] . u_c for its 8192 rows, via one fused DVE
    scalar_tensor_tensor (multiply + row-sum accumulator) per
    [128, 512] block -- the DVE-optimal form (fp32 tensor_tensor has no
    2x mode; the fused op does mult+reduce in one 1x pass).
  - ONE AllGather (32KB/rank) at the end of the pipeline collects all
    partials; each core sums its 4 column partials per row-group with a
    few small DVE adds.  There is no pre-compute collective, so the
    runtime's ~35us inter-core start barrier and the ~11us ncfw trigger
    latency hide entirely under the DMA/DVE phase.
  - Every core then does the softmax redundantly and writes the full
    [16384] result; host takes core 0's copy.  Cross-partition max via
    PE transpose + row reduce, cross-partition sum via matmul with a
    ones vector, scalar broadcasts via rank-1 matmul.
"""

import numpy as np

S = 16384
H = 2048
NCORES = 8
RG = 2  # row groups
CG = 4  # column groups
S_LOC = S // RG  # 8192 seq rows per core
H_SH = H // CG  # 512 enc/W columns per core
P = 128
NSUB = 4  # row-chunks per DMA tile
NT = S_LOC // (P * NSUB)  # 16 enc DMA tiles of [128, 4, 512] per core
NO = H // P  # 16 contraction chunks for the u matvec
NWH = 4  # wh DMA split for earlier matvec start
CHUNK = S_LOC // P  # 64 e elements per partition

_CACHE = {}


def _build_program():
    import concourse.bacc as bacc
    import concourse.mybir as mybir
    import concourse.tile as tile

    fp32 = mybir.dt.float32
    # Bacc (not raw Bass): its compile() splits multi-sem waits into
    # EventSemaphores and moves matmul waits onto ldweights -- TRN2
    # instructions carry at most one sync wait.
    nc = bacc.Bacc("TRN2")

    enc_in = nc.dram_tensor("enc", [S_LOC, H_SH], fp32, kind="ExternalInput")
    # packed per-core weights: wh[p, o, 0:H_SH] = W[o*128+p, c-shard],
    # wh[p, o, H_SH] = hidden[o*128+p].
    wh_in = nc.dram_tensor("wh", [P, NO, H_SH + 1], fp32, kind="ExternalInput")
    attn_out = nc.dram_tensor("attn", [S], fp32, kind="ExternalOutput")

    ident_dram = nc.inline_tensor(np.eye(P, dtype=np.float32), name="ident128")

    groups = [list(range(NCORES))]

    with tile.TileContext(nc) as tc:
        with (
            tc.tile_pool(name="const", bufs=1) as cpool,
            tc.tile_pool(name="encp", bufs=4) as enc_pool,
            tc.tile_pool(name="small", bufs=1) as small,
            tc.tile_pool(name="psum", bufs=1, space="PSUM") as psum,
            tc.tile_pool(name="dram", bufs=1, space="DRAM") as dram,
        ):
            e_part = dram.tile([S_LOC], fp32)
            e_ag = dram.tile([NCORES * S_LOC], fp32, addr_space="Shared")

            # ---- constants ----
            ident = cpool.tile([P, P], fp32)
            nc.scalar.dma_start(ident[:], ident_dram[:])
            ones_row = cpool.tile([1, P], fp32)  # [K=1, M=128] lhsT: bcast
            nc.vector.memset(ones_row[:], 1.0)
            neg_ones_row = cpool.tile([1, P], fp32)  # bcast with negate
            nc.vector.memset(neg_ones_row[:], -1.0)
            ones_col = cpool.tile([P, 1], fp32)  # [K=128, M=1] lhsT: P-sum
            nc.vector.memset(ones_col[:], 1.0)

            # ---- u_c = hidden @ W[:, c-shard] on the PE (local) ----
            # wh as NWH separate tiles so each matmul group starts as soon
            # as its DMA chunk lands (one tile would gate on the last chunk).
            OG = NO // NWH
            wh_tiles = []
            for w in range(NWH):
                wh_t = cpool.tile([P, OG, H_SH + 1], fp32, name=f"wh_t{w}")
                nc.scalar.dma_start(wh_t[:], wh_in[:, w * OG : (w + 1) * OG, :])
                wh_tiles.append(wh_t)
            u_ps = psum.tile([1, H_SH], fp32)
            for o in range(NO):
                wh_t = wh_tiles[o // OG]
                nc.tensor.matmul(
                    u_ps[:],
                    wh_t[:, o % OG, H_SH : H_SH + 1],
                    wh_t[:, o % OG, 0:H_SH],
                    start=(o == 0),
                    stop=(o == NO - 1),
                )
            u_sb = small.tile([1, H_SH], fp32)
            nc.scalar.copy(u_sb[:], u_ps[:])
            # broadcast u_c to all 128 partitions via rank-1 matmul; the stt
            # loop reads it straight from PSUM (fp32 DVE is 1x either way)
            ub_ps = psum.tile([P, H_SH], fp32)
            nc.tensor.matmul(ub_ps[:], ones_row[:], u_sb[:])

            # ---- partial energies for the core's 8192 rows ----
            # Row p*CHUNK + t*NSUB + m sits at (tile t, partition p, sub m):
            # e_psb[p, t*NSUB+m], so the e_part store is contiguous per
            # partition and the AllGather output keeps a regular layout.
            e_psb = small.tile([P, CHUNK], fp32)
            scratch = small.tile([P, H_SH], fp32)
            enc_r = enc_in.rearrange("(p t m) h -> t p m h", p=P, t=NT, m=NSUB)
            for t in range(NT):
                enc_t = enc_pool.tile([P, NSUB, H_SH], fp32, tag="enc_t")
                nc.sync.dma_start(enc_t[:], enc_r[t])
                for m in range(NSUB):
                    nc.vector.scalar_tensor_tensor(
                        out=scratch[:],
                        in0=enc_t[:, m, :],
                        scalar=1.0,
                        in1=ub_ps[:],
                        op0=mybir.AluOpType.mult,
                        op1=mybir.AluOpType.mult,
                        accum_out=e_psb[:, t * NSUB + m : t * NSUB + m + 1],
                    )
            nc.gpsimd.dma_start(e_part[:].rearrange("(p c) -> p c", p=P), e_psb[:])
            nc.gpsimd.collective_compute(
                "AllGather",
                mybir.AluOpType.bypass,
                replica_groups=groups,
                ins=[e_part[:]],
                outs=[e_ag[:]],
            )

            # ---- combine column partials, then softmax (redundant) ----
            # e_ag = (r p c): rank r = g*4+c holds rows g*8192 + p*64 + c'.
            parts = small.tile([P, NCORES, CHUNK], fp32)
            nc.gpsimd.dma_start(
                parts[:], e_ag[:].rearrange("(r p c) -> p r c", r=NCORES, p=P)
            )
            # ea[p, j]: j in [0,64) -> s = p*64 + j (row-group 0),
            #           j in [64,128) -> s = 8192 + p*64 + (j-64).
            # Pairwise tree sum over each row-group's 4 column partials.
            ea = small.tile([P, S // P], fp32)
            q = small.tile([P, NCORES // 2, CHUNK], fp32)
            parts_v = parts[:].rearrange("p (r2 b) c -> p r2 b c", b=2)
            nc.vector.tensor_add(q[:], parts_v[:, :, 0, :], parts_v[:, :, 1, :])
            q_v = q[:].rearrange("p (g b) c -> p g b c", b=2)
            nc.vector.tensor_add(
                ea[:].rearrange("p (g c) -> p g c", g=RG),
                q_v[:, :, 0, :],
                q_v[:, :, 1, :],
            )
            mx = small.tile([P, 1], fp32)
            nc.vector.reduce_max(mx[:], ea[:], axis=mybir.AxisListType.X)
            # global max: transpose [128,1] -> [1,128] on PE, reduce row 0
            mrow_ps = psum.tile([1, P], fp32)
            nc.tensor.transpose(mrow_ps[:], mx[:], ident[:])
            gmax = small.tile([1, 1], fp32)
            nc.vector.reduce_max(gmax[:], mrow_ps[:], axis=mybir.AxisListType.X)
            # broadcast -gmax to [128,1] (negated ones fold the sign)
            gb_ps = psum.tile([P, 1], fp32)
            nc.tensor.matmul(gb_ps[:], neg_ones_row[:], gmax[:])
            nmx = small.tile([P, 1], fp32)
            nc.scalar.copy(nmx[:], gb_ps[:])
            # exp(e - gmax) with per-partition row sums in one ACT op
            xs = small.tile([P, S // P], fp32)
            sums = small.tile([P, 1], fp32)
            nc.scalar.activation(
                xs[:],
                ea[:],
                mybir.ActivationFunctionType.Exp,
                bias=nmx[:],
                scale=1.0,
                accum_out=sums[:],
            )
            # global sum: contract the partition axis on the PE
            tot_ps = psum.tile([1, 1], fp32)
            nc.tensor.matmul(tot_ps[:], ones_col[:], sums[:])
            rec = small.tile([1, 1], fp32)
            nc.vector.reciprocal(rec[:], tot_ps[:])
            rb_ps = psum.tile([P, 1], fp32)
            nc.tensor.matmul(rb_ps[:], ones_row[:], rec[:])
            outx = small.tile([P, S // P], fp32)
            nc.vector.tensor_scalar_mul(outx[:], xs[:], rb_ps[:])
            # j in [0,64) -> s = p*64+j; j in [64,128) -> s = 8192+p*64+j-64
            nc.sync.dma_start(
                attn_out.rearrange("(a p c) -> p a c", a=RG, p=P),
                outx[:].rearrange("p (a c) -> p a c", a=RG),
            )

    nc.compile()
    return nc


def _get_program():
    if "nc" not in _CACHE:
        _CACHE["nc"] = _build_program()
    return _CACHE["nc"]


def _make_in_maps(hidden, encoder_outputs, W):
    hidden = np.ascontiguousarray(np.asarray(hidden, dtype=np.float32))
    enc = np.ascontiguousarray(np.asarray(encoder_outputs, dtype=np.float32))
    W = np.ascontiguousarray(np.asarray(W, dtype=np.float32))
    hid = hidden.reshape(NO, P).transpose(1, 0)  # hid[p, o] = hidden[o*128+p]
    # W as [p, o, h]: W_poh[p, o, h] = W[o*128+p, h]
    W_poh = W.reshape(NO, P, H).transpose(1, 0, 2)
    in_maps = []
    for r in range(NCORES):
        g, c = divmod(r, CG)
        wh = np.empty((P, NO, H_SH + 1), dtype=np.float32)
        wh[:, :, 0:H_SH] = W_poh[:, :, c * H_SH : (c + 1) * H_SH]
        wh[:, :, H_SH] = hid
        in_maps.append(
            {
                "enc": np.ascontiguousarray(
                    enc[g * S_LOC : (g + 1) * S_LOC, c * H_SH : (c + 1) * H_SH]
                ),
                "wh": wh,
            }
        )
    return in_maps


def run(hidden, encoder_outputs, W, b=None, trace=False):
    from concourse.bass_utils import run_bass_kernel_spmd

    nc = _get_program()
    in_maps = _make_in_maps(hidden, encoder_outputs, W)
    res = run_bass_kernel_spmd(nc, in_maps, list(range(NCORES)), trace=trace)
    out = np.asarray(res.results[0]["attn"], dtype=np.float32).reshape(1, 1, S)
    return out, res


def kernel(hidden, encoder_outputs, W, b):
    out, _ = run(hidden, encoder_outputs, W, b)
    return out


# revision 18
# speedup vs baseline: 1.2640x; 1.2640x over previous
"""Trainium2 Bass kernel for nn_Attn: attn = softmax(enc @ W^T @ hidden^T).

Math: reference computes energy = enc @ W^T + b  ([S,H]), then
attn_energies = energy @ hidden[0]  ([S]), then softmax over S.
Associativity: attn_energies = enc @ (W^T @ hidden^T) + (b . hidden).
The (b . hidden) term is a constant shift over S -> softmax-invariant
(and b is zeros for this problem), so we drop it.

Distribution over 8 cores = 2 row-groups x 4 column-groups:
  core r -> row-group g = r // 4 (8192 seq rows), col-group c = r % 4
  (512 hidden columns).
  - Each core computes u_c = hidden @ W[:, c-shard] on the PE (purely
    local; W shard is 4MB) and then partial energies
    e_r[s] = enc[s, c-shard] . u_c for its 8192 rows, via one fused DVE
    scalar_tensor_tensor (multiply + row-sum accumulator) per
    [128, 512] block -- the DVE-optimal form (fp32 tensor_tensor has no
    2x mode; the fused op does mult+reduce in one 1x pass).
  - ONE AllGather (32KB/rank) at the end of the pipeline collects all
    partials; each core sums its 4 column partials per row-group with a
    few small DVE adds.  There is no pre-compute collective, so the
    runtime's ~35us inter-core start barrier and the ~11us ncfw trigger
    latency hide entirely under the DMA/DVE phase.
  - Every core then does the softmax redundantly and writes the full
    [16384] result; host takes core 0's copy.  Cross-partition max via
    PE transpose + row reduce, cross-partition sum via matmul with a
    ones vector, scalar broadcasts via rank-1 matmul.
"""

import numpy as np

S = 16384
H = 2048
NCORES = 8
RG = 2  # row groups
CG = 4  # column groups
S_LOC = S // RG  # 8192 seq rows per core
H_SH = H // CG  # 512 enc/W columns per core
P = 128
NSUB = 4  # row-chunks per DMA tile
NT = S_LOC // (P * NSUB)  # 16 enc DMA tiles of [128, 4, 512] per core
NO = H // P  # 16 contraction chunks for the u matvec
NWH = 8  # wh DMA split for earlier matvec start
CHUNK = S_LOC // P  # 64 e elements per partition

_CACHE = {}


def _build_program():
    import concourse.bacc as bacc
    import concourse.mybir as mybir
    import concourse.tile as tile

    fp32 = mybir.dt.float32
    # Bacc (not raw Bass): its compile() splits multi-sem waits into
    # EventSemaphores and moves matmul waits onto ldweights -- TRN2
    # instructions carry at most one sync wait.
    nc = bacc.Bacc("TRN2")

    enc_in = nc.dram_tensor("enc", [S_LOC, H_SH], fp32, kind="ExternalInput")
    # packed per-core weights: wh[p, o, 0:H_SH] = W[o*128+p, c-shard],
    # wh[p, o, H_SH] = hidden[o*128+p].
    wh_in = nc.dram_tensor("wh", [P, NO, H_SH + 1], fp32, kind="ExternalInput")
    attn_out = nc.dram_tensor("attn", [S], fp32, kind="ExternalOutput")

    ident_dram = nc.inline_tensor(np.eye(P, dtype=np.float32), name="ident128")

    groups = [list(range(NCORES))]

    with tile.TileContext(nc) as tc:
        with (
            tc.tile_pool(name="const", bufs=1) as cpool,
            tc.tile_pool(name="encp", bufs=4) as enc_pool,
            tc.tile_pool(name="small", bufs=1) as small,
            tc.tile_pool(name="psum", bufs=1, space="PSUM") as psum,
            tc.tile_pool(name="dram", bufs=1, space="DRAM") as dram,
        ):
            e_part = dram.tile([S_LOC], fp32)
            e_ag = dram.tile([NCORES * S_LOC], fp32, addr_space="Shared")

            # ---- constants ----
            ident = cpool.tile([P, P], fp32)
            nc.scalar.dma_start(ident[:], ident_dram[:])
            ones_row = cpool.tile([1, P], fp32)  # [K=1, M=128] lhsT: bcast
            nc.vector.memset(ones_row[:], 1.0)
            neg_ones_row = cpool.tile([1, P], fp32)  # bcast with negate
            nc.vector.memset(neg_ones_row[:], -1.0)
            ones_col = cpool.tile([P, 1], fp32)  # [K=128, M=1] lhsT: P-sum
            nc.vector.memset(ones_col[:], 1.0)

            # ---- u_c = hidden @ W[:, c-shard] ----
            # Per-partition-scalar multiply-accumulate on the DVE (fp32 PE
            # matmuls are dual-pass and took ~20us serial); each op handles
            # one 128-row chunk of d as soon as its wh DMA chunk lands.
            # Then ONE ones-matmul on the PE does the cross-partition sum
            # AND the broadcast to all 128 partitions; the stt loop reads
            # the result straight from PSUM.
            ones_mat = cpool.tile([P, P], fp32)
            nc.vector.memset(ones_mat[:], 1.0)
            OG = NO // NWH
            wh_tiles = []
            for w in range(NWH):
                wh_t = cpool.tile([P, OG, H_SH + 1], fp32, name=f"wh_t{w}")
                nc.scalar.dma_start(wh_t[:], wh_in[:, w * OG : (w + 1) * OG, :])
                wh_tiles.append(wh_t)
            u_acc = small.tile([P, H_SH], fp32)
            nc.vector.memset(u_acc[:], 0.0)
            for o in range(NO):
                wh_t = wh_tiles[o // OG]
                nc.vector.scalar_tensor_tensor(
                    out=u_acc[:],
                    in0=wh_t[:, o % OG, 0:H_SH],
                    scalar=wh_t[:, o % OG, H_SH : H_SH + 1],
                    in1=u_acc[:],
                    op0=mybir.AluOpType.mult,
                    op1=mybir.AluOpType.add,
                )
            ub_ps = psum.tile([P, H_SH], fp32)
            nc.tensor.matmul(ub_ps[:], ones_mat[:], u_acc[:])

            # ---- partial energies for the core's 8192 rows ----
            # Row p*CHUNK + t*NSUB + m sits at (tile t, partition p, sub m):
            # e_psb[p, t*NSUB+m], so the e_part store is contiguous per
            # partition and the AllGather output keeps a regular layout.
            e_psb = small.tile([P, CHUNK], fp32)
            scratch = small.tile([P, H_SH], fp32)
            enc_r = enc_in.rearrange("(p t m) h -> t p m h", p=P, t=NT, m=NSUB)
            for t in range(NT):
                enc_t = enc_pool.tile([P, NSUB, H_SH], fp32, tag="enc_t")
                nc.sync.dma_start(enc_t[:], enc_r[t])
                for m in range(NSUB):
                    nc.vector.scalar_tensor_tensor(
                        out=scratch[:],
                        in0=enc_t[:, m, :],
                        scalar=1.0,
                        in1=ub_ps[:],
                        op0=mybir.AluOpType.mult,
                        op1=mybir.AluOpType.mult,
                        accum_out=e_psb[:, t * NSUB + m : t * NSUB + m + 1],
                    )
            nc.gpsimd.dma_start(e_part[:].rearrange("(p c) -> p c", p=P), e_psb[:])
            nc.gpsimd.collective_compute(
                "AllGather",
                mybir.AluOpType.bypass,
                replica_groups=groups,
                ins=[e_part[:]],
                outs=[e_ag[:]],
            )

            # ---- combine column partials, then softmax (redundant) ----
            # e_ag = (r p c): rank r = g*4+c holds rows g*8192 + p*64 + c'.
            parts = small.tile([P, NCORES, CHUNK], fp32)
            nc.gpsimd.dma_start(
                parts[:], e_ag[:].rearrange("(r p c) -> p r c", r=NCORES, p=P)
            )
            # ea[p, j]: j in [0,64) -> s = p*64 + j (row-group 0),
            #           j in [64,128) -> s = 8192 + p*64 + (j-64).
            # Pairwise tree sum over each row-group's 4 column partials.
            ea = small.tile([P, S // P], fp32)
            q = small.tile([P, NCORES // 2, CHUNK], fp32)
            parts_v = parts[:].rearrange("p (r2 b) c -> p r2 b c", b=2)
            nc.vector.tensor_add(q[:], parts_v[:, :, 0, :], parts_v[:, :, 1, :])
            q_v = q[:].rearrange("p (g b) c -> p g b c", b=2)
            nc.vector.tensor_add(
                ea[:].rearrange("p (g c) -> p g c", g=RG),
                q_v[:, :, 0, :],
                q_v[:, :, 1, :],
            )
            mx = small.tile([P, 1], fp32)
            nc.vector.reduce_max(mx[:], ea[:], axis=mybir.AxisListType.X)
            # global max: transpose [128,1] -> [1,128] on PE, reduce row 0
            mrow_ps = psum.tile([1, P], fp32)
            nc.tensor.transpose(mrow_ps[:], mx[:], ident[:])
            gmax = small.tile([1, 1], fp32)
            nc.vector.reduce_max(gmax[:], mrow_ps[:], axis=mybir.AxisListType.X)
            # broadcast -gmax to [128,1] (negated ones fold the sign)
            gb_ps = psum.tile([P, 1], fp32)
            nc.tensor.matmul(gb_ps[:], neg_ones_row[:], gmax[:])
            nmx = small.tile([P, 1], fp32)
            nc.scalar.copy(nmx[:], gb_ps[:])
            # exp(e - gmax) with per-partition row sums in one ACT op
            xs = small.tile([P, S // P], fp32)
            sums = small.tile([P, 1], fp32)
            nc.scalar.activation(
                xs[:],
                ea[:],
                mybir.ActivationFunctionType.Exp,
                bias=nmx[:],
                scale=1.0,
                accum_out=sums[:],
            )
            # global sum: contract the partition axis on the PE
            tot_ps = psum.tile([1, 1], fp32)
            nc.tensor.matmul(tot_ps[:], ones_col[:], sums[:])
            rec = small.tile([1, 1], fp32)
            nc.vector.reciprocal(rec[:], tot_ps[:])
            rb_ps = psum.tile([P, 1], fp32)
            nc.tensor.matmul(rb_ps[:], ones_row[:], rec[:])
            outx = small.tile([P, S // P], fp32)
            nc.vector.tensor_scalar_mul(outx[:], xs[:], rb_ps[:])
            # j in [0,64) -> s = p*64+j; j in [64,128) -> s = 8192+p*64+j-64
            nc.sync.dma_start(
                attn_out.rearrange("(a p c) -> p a c", a=RG, p=P),
                outx[:].rearrange("p (a c) -> p a c", a=RG),
            )

    nc.compile()
    return nc


def _get_program():
    if "nc" not in _CACHE:
        _CACHE["nc"] = _build_program()
    return _CACHE["nc"]


def _make_in_maps(hidden, encoder_outputs, W):
    hidden = np.ascontiguousarray(np.asarray(hidden, dtype=np.float32))
    enc = np.ascontiguousarray(np.asarray(encoder_outputs, dtype=np.float32))
    W = np.ascontiguousarray(np.asarray(W, dtype=np.float32))
    hid = hidden.reshape(NO, P).transpose(1, 0)  # hid[p, o] = hidden[o*128+p]
    # W as [p, o, h]: W_poh[p, o, h] = W[o*128+p, h]
    W_poh = W.reshape(NO, P, H).transpose(1, 0, 2)
    in_maps = []
    for r in range(NCORES):
        g, c = divmod(r, CG)
        wh = np.empty((P, NO, H_SH + 1), dtype=np.float32)
        wh[:, :, 0:H_SH] = W_poh[:, :, c * H_SH : (c + 1) * H_SH]
        wh[:, :, H_SH] = hid
        in_maps.append(
            {
                "enc": np.ascontiguousarray(
                    enc[g * S_LOC : (g + 1) * S_LOC, c * H_SH : (c + 1) * H_SH]
                ),
                "wh": wh,
            }
        )
    return in_maps


def run(hidden, encoder_outputs, W, b=None, trace=False):
    from concourse.bass_utils import run_bass_kernel_spmd

    nc = _get_program()
    in_maps = _make_in_maps(hidden, encoder_outputs, W)
    res = run_bass_kernel_spmd(nc, in_maps, list(range(NCORES)), trace=trace)
    out = np.asarray(res.results[0]["attn"], dtype=np.float32).reshape(1, 1, S)
    return out, res


def kernel(hidden, encoder_outputs, W, b):
    out, _ = run(hidden, encoder_outputs, W, b)
    return out
